# revision 23
# baseline (speedup 1.0000x reference)
"""GQA (grouped-query attention) Trainium2 kernel, tensor-parallel over 8 NeuronCores.

Sharding: core c computes query heads {2c, 2c+1} and kv head c//2 (groups kept
intact), with wo row-sharded; each core returns a partial [B*S, HID] output and
the host sums the 8 partials.

Per-core device kernel (matmuls in bf16, f32 PSUM accumulate; rel err ~5e-3):
  phase 1: Q^T/K^T/V^T projections from host-relayouted bf16 inputs, RoPE on
           Q/K (DVE products + GpSimd add), V to natural layout via f32 PE
           transpose. All DRAM operands are partition-major [128, ...] so each
           DMA is one contiguous run per partition: HWDGE issue cost scales
           with descriptor-row count (a strided rearrange DMA measured 15.9us
           of queue-issue time and starved the PE for the whole first ramp).
           x loads as one 2MB DMA per 512-token block (ko0/1-3/4-15 split for
           block 0 so the first matmul starts ~1us in); weight chunk 0 lands
           first, the rest as one DMA each; rope tables are batch-independent
           [128, S] halves indexed s0 % S; the 1MB wo load is deferred past
           the DMA-congested ramp.
  phase 2: per (batch, 512-wide q-block, head): scores^T = K_blk Q^T (PE) with
           the causal mask folded in as a -3e4 constant-add matmul on the
           diagonal blocks -> exp (ACT, key_weight*scale folded into the
           activation scale; assumes key_weights > 0) -> attn^T accum +
           sum-of-exp via ones-matmul (PE, denominator at partition 0: PE PSUM
           writes at partition offset 32 do not land correctly). Softmax
           normalization: per-head reciprocal_approx_fast (DVE) +
           partition_broadcast (GpSimd) + attn^T * bcast (DVE). wo projection
           consumes attn^T directly and is emitted one block late so its
           matmuls fill the next block's exp-latency bubbles; PSUM->SBUF
           copies round-robin over DVE/ACT assemble full 2048-wide rows in
           SBUF so each output store is one 1MB DMA per 128-row block.

Schedule notes from this round of optimization (trn2, measured via NTFF):
  - 361.1us from a 433us starting point; PE (TensorMatrix) is the bottleneck
    engine at ~84-86% busy, ACT ~47% (exp dominates), DVE ~46%. Remaining
    idle: ~12us DMA-bandwidth-bound ramp (7MB of x/weights/consts must land
    before block 1 finishes), ~3us phase transition, ~8us scattered, plus
    ~17us HAM cold-clock penalty concentrated in the ramp. The final drain
    stores each 512-col strip as its PSUM copy lands (emit_wo final=True).
  - Tried and REVERTED: interleaving projections with attention under
    coexisting 8-bank PSUM pools (passA q0/q1 -> attn half -> passB k/v), and
    a score-one-block-ahead rotation with per-jj wo filler. Both are
    CoreSim-clean but intermittently crash (NRT_EXEC_UNIT_UNRECOVERABLE) or
    corrupt results on hardware - consistent with a cross-engine semaphore /
    PSUM-collision hazard that Tile does not fully guard at this interleave
    density. Keep the two-phase structure and head-boundary emit_wo(8) bursts;
    a per-jj emit_wo(1) filler also measured SLOWER (375us).
"""
import numpy as np
import ml_dtypes

BF16 = ml_dtypes.bfloat16

B, S, HID = 2, 2048, 2048
NH, NKV, D = 16, 4, 128
NCORES = 8
HPC = NH // NCORES            # q heads per core
SQ = B * S                    # 4096 tokens
NKO = HID // 128              # 16 contraction chunks
NSQB = SQ // 512              # 8 sequence blocks of 512
JPB = S // 512                # 4 q-blocks per batch
KPB = S // 128                # 16 k-blocks per batch
ROPE_BASE = 10000.0
SCALE = float(D) ** -0.5
# mask constant: large enough to dominate any score (|s| < ~100), small enough
# that the ACT exp's integer range-reduction doesn't overflow (exp(-1e38)
# returns garbage on HW; exp(-2652) underflows cleanly to 0)
NEG = -30000.0

_cache = {}


def _consts():
    half = D // 2
    pos = np.arange(S, dtype=np.float32)
    inv_freq = (1.0 / (ROPE_BASE ** (np.arange(half, dtype=np.float32) / np.float32(half)))).astype(np.float32)
    ang = pos[:, None] * inv_freq[None, :]              # [S, 64]
    cos = np.cos(ang).astype(np.float32).T              # [64, S]
    sin = np.sin(ang).astype(np.float32).T
    # [128, S]: identical for every batch, so index with s0 % S (halves DMA)
    cos_full = np.concatenate([cos, cos], 0).astype(BF16)
    sinpm = np.concatenate([-sin, sin], 0).astype(BF16)
    r = np.arange(128)
    # maskA[r, c] = 0 if c >= r else NEG   (within-block causal)
    maskA = np.where(r[None, :] >= r[:, None], 0.0, NEG).astype(BF16)        # [128,128]
    ident = np.eye(128, dtype=np.float32).astype(BF16)
    identf = np.eye(128, dtype=np.float32)
    ones_col = np.ones((128, 1), np.float32).astype(BF16)
    return cos_full, sinpm, maskA, ident, identf, ones_col


def _build(phases=(1, 2)):
    import concourse.mybir as mybir
    from concourse import bacc
    from concourse.tile import TileContext
    from contextlib import ExitStack

    f32 = mybir.dt.float32
    bf16 = mybir.dt.bfloat16
    MUL = mybir.AluOpType.mult
    ADD = mybir.AluOpType.add
    EXP = mybir.ActivationFunctionType.Exp
    CPY = mybir.ActivationFunctionType.Copy

    cos_np, sinpm_np, maskA_np, ident_np, identf_np, onescol_np = _consts()

    nc = bacc.Bacc("TRN2", target_bir_lowering=False, debug=False)

    # x relayout (host-side): xh[p, sb, ko, s] = x[sb*512+s, ko*128+p] so one
    # 512-token block is a single contiguous 2MB DMA (16KB/partition)
    xh = nc.dram_tensor("xh", [128, NSQB * NKO * 512], bf16, kind="ExternalInput")
    # weights host-relayouted to [128, ...] partition-major so every weight DMA
    # is one contiguous run per partition (HWDGE issue cost scales with
    # descriptor-row count: a strided rearrange DMA measured 15.9us to issue)
    wqh = nc.dram_tensor("wqh", [128, NKO * HPC * D], bf16, kind="ExternalInput")
    wkh = nc.dram_tensor("wkh", [128, NKO * D], bf16, kind="ExternalInput")
    wvh = nc.dram_tensor("wvh", [128, NKO * D], bf16, kind="ExternalInput")
    woh = nc.dram_tensor("woh", [128, HPC * HID], bf16, kind="ExternalInput")
    kw = nc.dram_tensor("kw", [HPC], f32, kind="ExternalInput")
    out = nc.dram_tensor("out", [SQ, HID], f32, kind="ExternalOutput")

    cos_d = nc.inline_tensor(cos_np, name="cos_t")
    sinpm_d = nc.inline_tensor(sinpm_np, name="sinpm_t")
    maskA_d = nc.inline_tensor(maskA_np, name="maskA_t")
    ident_d = nc.inline_tensor(ident_np, name="ident_t")
    identf_d = nc.inline_tensor(identf_np, name="identf_t")
    onescol_d = nc.inline_tensor(onescol_np, name="onescol_t")

    with TileContext(nc) as tc:
        with tc.tile_pool(name="persist", bufs=1) as pp:
            # persistent SBUF tensors (all matmul operands bf16)
            wq_sb = pp.tile([128, NKO, HPC * D], bf16, tag="wq")
            wk_sb = pp.tile([128, NKO, D], bf16, tag="wk")
            wv_sb = pp.tile([128, NKO, D], bf16, tag="wv")
            wo_sb = pp.tile([128, HPC, HID], bf16, tag="wo")
            q_sb = [pp.tile([128, SQ], bf16, tag=f"q{h}", name=f"q_sb{h}") for h in range(HPC)]
            k_sb = pp.tile([128, SQ], bf16, tag="k")
            # one contiguous [128,128] tile per V k-block: dma_start_transpose
            # corrupts non-contiguous (sliced) destinations on hardware
            v_sb = [pp.tile([128, D], bf16, tag=f"v{i}", name=f"v_sb{i}")
                    for i in range(SQ // 128)]
            cos_sb = pp.tile([128, S], bf16, tag="cos")
            sinpm_sb = pp.tile([128, S], bf16, tag="sinpm")
            maskA_sb = pp.tile([128, 128], bf16, tag="maskA")
            ident_sb = pp.tile([128, 128], bf16, tag="ident")
            identf_sb = pp.tile([128, 128], f32, tag="identf")
            onescol_sb = pp.tile([128, 1], bf16, tag="onescol")
            kwsc_sb = pp.tile([128, HPC], f32, tag="kwsc")
            kwraw_sb = pp.tile([128, HPC], f32, tag="kwraw")

            es = ExitStack()
            rt = es.enter_context(tc.tile_pool(name="rt", bufs=2))
            vs = es.enter_context(tc.tile_pool(name="vs", bufs=4))
            ep = es.enter_context(tc.tile_pool(name="ep", bufs=3))
            ap = es.enter_context(tc.tile_pool(name="ap", bufs=4))
            bp = es.enter_context(tc.tile_pool(name="bp", bufs=2))
            smp = es.enter_context(tc.tile_pool(name="smp", bufs=4))
            fsb = es.enter_context(tc.tile_pool(name="fsb", bufs=4))

            def rope(dst, src_ps, s0):
                s0 = s0 % S  # rope tables are per-position, batch-independent
                t0 = rt.tile([128, 512], f32, tag="t0")
                t1 = rt.tile([128, 512], f32, tag="t1")
                nc.vector.tensor_tensor(t0[:], src_ps[:], cos_sb[:, s0:s0 + 512], MUL)
                nc.vector.tensor_tensor(t1[0:64, :], src_ps[64:128, :], sinpm_sb[0:64, s0:s0 + 512], MUL)
                nc.vector.tensor_tensor(t1[64:128, :], src_ps[0:64, :], sinpm_sb[64:128, s0:s0 + 512], MUL)
                # final add on GpSimd: it is idle, and it does not read the PSUM
                # source, so the qps/kps slot frees as soon as t0/t1 are read
                nc.gpsimd.tensor_tensor(dst, t0[:], t1[:], ADD)

            def proj_block(pps, vpsp, tps, xp, sb):
                s0 = sb * 512
                qps = [pps.tile([128, 512], f32, tag=f"qps{h}", name=f"qps{h}") for h in range(HPC)]
                kps = pps.tile([128, 512], f32, tag="kps", name="kps")
                vps = vpsp.tile([128, 512], f32, tag="vps", name="vps")
                if sb == 0:
                    # weights: ko-chunk 0 small (fast start), rest as one big
                    # contiguous DMA each -- per-ko issues serialized the
                    # scalar HWDGE queue (~30us) and starved the first ramp
                    wqr = wqh[:].rearrange("p (ko d) -> p ko d", ko=NKO)
                    wkr = wkh[:].rearrange("p (ko d) -> p ko d", ko=NKO)
                    wvr = wvh[:].rearrange("p (ko d) -> p ko d", ko=NKO)
                    nc.scalar.dma_start(wq_sb[:, 0, :], wqr[:, 0, :])
                    nc.scalar.dma_start(wk_sb[:, 0, :], wkr[:, 0, :])
                    nc.scalar.dma_start(wv_sb[:, 0, :], wvr[:, 0, :])
                    nc.scalar.dma_start(wq_sb[:, 1:NKO, :], wqr[:, 1:NKO, :])
                    nc.scalar.dma_start(wk_sb[:, 1:NKO, :], wkr[:, 1:NKO, :])
                    nc.scalar.dma_start(wv_sb[:, 1:NKO, :], wvr[:, 1:NKO, :])
                    nc.scalar.dma_start(identf_sb[:], identf_d[:])
                    nc.scalar.dma_start(cos_sb[:], cos_d[:])
                    nc.scalar.dma_start(sinpm_sb[:], sinpm_d[:])
                    nc.scalar.dma_start(kwraw_sb[:], kw[None, :].to_broadcast((128, HPC)))
                    nc.vector.tensor_scalar_mul(kwsc_sb[:], kwraw_sb[:], SCALE)
                xt = xp.tile([128, NKO, 512], bf16, tag="x")
                xsrc = xh[:].rearrange("p (sb ko s) -> p sb ko s", sb=NSQB, ko=NKO)
                if sb == 0:
                    # split so the first matmul starts ~1us in
                    nc.sync.dma_start(xt[:, 0, :], xsrc[:, 0, 0, :])
                    nc.sync.dma_start(xt[:, 1:4, :], xsrc[:, 0, 1:4, :])
                    nc.sync.dma_start(xt[:, 4:NKO, :], xsrc[:, 0, 4:NKO, :])
                else:
                    nc.sync.dma_start(xt[:], xsrc[:, sb])
                for ko in range(NKO):
                    st, sp = (ko == 0), (ko == NKO - 1)
                    for h in range(HPC):
                        nc.tensor.matmul(qps[h][:], wq_sb[:, ko, h * D:(h + 1) * D], xt[:, ko, :], start=st, stop=sp)
                    nc.tensor.matmul(kps[:], wk_sb[:, ko, :], xt[:, ko, :], start=st, stop=sp)
                    nc.tensor.matmul(vps[:], wv_sb[:, ko, :], xt[:, ko, :], start=st, stop=sp)
                if sb == 1:
                    nc.scalar.dma_start(maskA_sb[:], maskA_d[:])
                    nc.scalar.dma_start(ident_sb[:], ident_d[:])
                    nc.scalar.dma_start(onescol_sb[:], onescol_d[:])
                if sb == 3:
                    nc.scalar.dma_start(wo_sb[:], woh[:].rearrange("p (h e) -> p h e", h=HPC))
                # V: one ACT copy frees the vps bank, then f32 PE transposes
                # [d, s] -> [s, d] (xbar DMA transposes serialized the scalar
                # queue at 1.1us each and gated the phase-1 pipeline; bf16
                # PSUM transpose staging produces NaN on HW, so stage in f32)
                vst = vs.tile([128, 512], f32, tag="vst", name="vst")
                nc.scalar.activation(vst[:], vps[:], CPY)
                for i in range(4):
                    vtp = tps.tile([128, 128], f32, tag="vt", name=f"vtp{i}")
                    nc.tensor.transpose(vtp[:], vst[:, i * 128:(i + 1) * 128], identf_sb[:])
                    nc.scalar.activation(v_sb[sb * 4 + i][:], vtp[:], CPY)
                for h in range(HPC):
                    rope(q_sb[h][:, s0:s0 + 512], qps[h], s0)
                rope(k_sb[:, s0:s0 + 512], kps, s0)

            if 1 not in phases:  # ablation-timing only: fabricate phase-1 outputs
                for h in range(HPC):
                    nc.gpsimd.memset(q_sb[h][:], 0.5)
                nc.gpsimd.memset(k_sb[:], 0.5)
                for t in v_sb:
                    nc.gpsimd.memset(t[:], 0.5)

            OFF = [0, 128, 256, 384]
            if 1 in phases:
                with tc.tile_pool(name="pps", bufs=2, space="PSUM") as pps, \
                     tc.tile_pool(name="vps_p", bufs=1, space="PSUM") as vpsp, \
                     tc.tile_pool(name="tps", bufs=1, space="PSUM") as tps, \
                     tc.tile_pool(name="xp", bufs=3) as xp:
                    for sb in range(NSQB):
                        proj_block(pps, vpsp, tps, xp, sb)

            p2 = ExitStack()
            if 2 in phases:
                scps = p2.enter_context(tc.tile_pool(name="scps", bufs=2, space="PSUM"))
                avps = p2.enter_context(tc.tile_pool(name="avps", bufs=2, space="PSUM"))
                sups = p2.enter_context(tc.tile_pool(name="sups", bufs=2, space="PSUM"))
                mfin = p2.enter_context(tc.tile_pool(name="mfin", bufs=2, space="PSUM"))

            # deferred wo work from the previous (b, J) block: emitting it
            # between the next block's score/exp matmuls keeps the PE warm
            # through the softmax finish
            pending = []
            fo_engines = [nc.vector, nc.scalar, nc.vector]  # gpsimd can't read PSUM
            fo_rr = [0]
            fo_state = {}

            def emit_wo(n, final=False):
                for _ in range(n):
                    if not pending:
                        return
                    q0w, ats, i, e = pending.pop(0)
                    fp = mfin.tile([128, 512], f32, tag="mf", name="fp")
                    for h in range(HPC):
                        nc.tensor.matmul(fp[:], ats[h][:, i * 128:(i + 1) * 128],
                                         wo_sb[:, h, e * 512:(e + 1) * 512],
                                         start=(h == 0), stop=(h == HPC - 1))
                    # assemble full 2048-wide output rows in SBUF; one 1MB DMA
                    # per 128-row block (4x fewer, descriptor-efficient stores)
                    if e == 0:
                        fo_state["fo"] = fsb.tile([128, HID], f32, tag="fo", bufs=2, name="fo")
                    fo = fo_state["fo"]
                    eng = fo_engines[fo_rr[0] % 3]
                    fo_rr[0] += 1
                    if eng is nc.scalar:
                        eng.activation(fo[:, e * 512:(e + 1) * 512], fp[:], CPY)
                    else:
                        eng.tensor_copy(fo[:, e * 512:(e + 1) * 512], fp[:])
                    if final:
                        # tail drain: store each 512-col strip as its copy
                        # lands so the last DMA carries 256KB, not 1MB
                        nc.sync.dma_start(out[q0w + i * 128:q0w + (i + 1) * 128,
                                              e * 512:(e + 1) * 512],
                                          fo[:, e * 512:(e + 1) * 512])
                    elif e == 3:
                        nc.sync.dma_start(out[q0w + i * 128:q0w + (i + 1) * 128, :], fo[:])

            for b in (range(B) if 2 in phases else ()):
                t0 = b * S
                for J in range(JPB):
                    q0 = t0 + J * 512
                    nkb = 4 * J + 4
                    attn_now = []
                    for h in range(HPC):
                        avp = avps.tile([128, 512], f32, tag="av", name="avp")
                        # per-head denominator at partition 0: PE matmul PSUM
                        # writes at partition offset 32 do not land correctly
                        sup = sups.tile([1, 512], f32, tag="su", name="sup")
                        for jj in range(nkb):
                            p = jj - 4 * J
                            off = OFF[p] if p >= 0 else 0
                            n = 512 - off
                            scp = scps.tile([128, 512], f32, tag="sc", name="scp")
                            diag = (p >= 0)
                            nc.tensor.matmul(
                                scp[:, 0:n],
                                k_sb[:, t0 + jj * 128:t0 + (jj + 1) * 128],
                                q_sb[h][:, q0 + off:q0 + 512],
                                start=True, stop=not diag)
                            if diag:
                                # causal mask: add -1e38 to the sub-diagonal of
                                # the block (valid for key_weights > 0)
                                nc.tensor.matmul(scp[:, 0:128], ident_sb[:],
                                                 maskA_sb[:], start=False, stop=True)
                            ex = ep.tile([128, 512], bf16, tag="ex")
                            nc.scalar.activation(ex[:, 0:n], scp[:, 0:n], EXP,
                                                 scale=kwsc_sb[:, h:h + 1])
                            st, sp = (jj == 0), (jj == nkb - 1)
                            nc.tensor.matmul(avp[:, off:512], v_sb[b * KPB + jj][:],
                                             ex[:, 0:n], start=st, stop=sp)
                            nc.tensor.matmul(sup[:, off:512], onescol_sb[:],
                                             ex[:, 0:n], start=st, stop=sp)
                        # normalization chain runs on DVE/GpSimd under the other
                        # head's (or deferred wo) PE work
                        recip = smp.tile([1, 512], f32, tag=f"recip{h}", name="recip")
                        nc.vector.reciprocal_approx_fast(recip[:], sup[:])
                        bcs = bp.tile([128, 512], f32, tag="bcs")
                        nc.gpsimd.partition_broadcast(bcs[:], recip[:])
                        at = ap.tile([128, 512], bf16, tag="at")
                        nc.vector.tensor_tensor(at[:], avp[:], bcs[:], MUL)
                        attn_now.append(at)
                        emit_wo(8)
                    emit_wo(len(pending))  # drain any leftovers (J=0 blocks)
                    ats = list(attn_now)
                    for i in range(4):
                        for e in range(4):
                            pending.append((q0, ats, i, e))
            emit_wo(len(pending), final=True)
            p2.close()
            es.close()

    nc.compile()
    return nc


def _get_exec():
    """Build the Bass module once and wrap it in a cached jitted shard_map
    executable (mirrors concourse.bass2jax.run_bass_via_pjrt, minus donation so
    repeated calls can reuse device-resident buffers)."""
    if "exec" in _cache:
        return _cache["exec"]
    import jax
    import concourse.mybir as mybir
    from jax.experimental.shard_map import shard_map
    from jax.sharding import Mesh, PartitionSpec
    from concourse import bass2jax

    nc = _build()
    bass2jax.install_neuronx_cc_hook()

    partition_name = nc.partition_id_tensor.name if nc.partition_id_tensor else None
    in_names, out_names, out_avals = [], [], []
    for alloc in nc.m.functions[0].allocations:
        if not isinstance(alloc, mybir.__dict__["MemoryLocationSet"]):
            continue
        name = alloc.memorylocations[0].name
        if alloc.kind == "ExternalInput":
            if name != partition_name:
                in_names.append(name)
        elif alloc.kind == "ExternalOutput":
            out_names.append(name)
            out_avals.append(jax.core.ShapedArray(tuple(alloc.tensor_shape),
                                                  mybir.dt.np(alloc.dtype)))
    n_params = len(in_names)
    in_names = in_names + out_names  # zero-buffer operands, per bass2jax contract
    if partition_name is not None:
        in_names.append(partition_name)

    def _body(*args):
        operands = list(args)
        if partition_name is not None:
            operands.append(bass2jax.partition_id_tensor())
        outs = bass2jax._bass_exec_p.bind(
            *operands,
            out_avals=tuple(out_avals),
            in_names=tuple(in_names),
            out_names=tuple(out_names),
            lowering_input_output_aliases=(),
            sim_require_finite=True,
            sim_require_nnan=True,
            nc=nc,
        )
        return tuple(outs)

    devices = jax.devices()[:NCORES]
    mesh = Mesh(np.asarray(devices), ("core",))
    spec = PartitionSpec("core")
    sharded = jax.jit(
        shard_map(_body, mesh=mesh,
                  in_specs=(spec,) * (n_params + len(out_names)),
                  out_specs=(spec,) * len(out_names),
                  check_rep=False),
        keep_unused=True,
    )
    _cache["nc"] = nc
    _cache["exec"] = {
        "sharded": sharded, "in_names": in_names, "out_names": out_names,
        "out_avals": out_avals, "n_params": n_params, "mesh": mesh, "spec": spec,
    }
    return _cache["exec"]


def _prep_in_maps(x, wq, wk, wv, wo, key_weights):
    x = np.ascontiguousarray(np.asarray(x, dtype=np.float32))
    wq = np.asarray(wq, dtype=np.float32)
    wk = np.asarray(wk, dtype=np.float32)
    wv = np.asarray(wv, dtype=np.float32)
    wo = np.asarray(wo, dtype=np.float32)
    key_weights = np.asarray(key_weights, dtype=np.float32)

    # xh[p, sb, ko, s] = x[sb*512+s, ko*128+p]: per 512-token block the load is
    # one contiguous 16KB/partition DMA
    xh = np.ascontiguousarray(
        x.reshape(NSQB, 512, NKO, 128).transpose(3, 0, 2, 1).reshape(128, -1).astype(BF16))
    wqT = wq.T.astype(BF16)                                        # [HID, NH*D]
    wkT = wk.T.astype(BF16)                                        # [HID, NKV*D]
    wvT = wv.T.astype(BF16)
    woT = wo.T.astype(BF16)                                        # [NH*D, HID]

    def pmaj(a):  # [n*128, m] -> [128, n*m]: partition-major, contiguous rows
        n = a.shape[0] // 128
        return np.ascontiguousarray(
            a.reshape(n, 128, a.shape[1]).transpose(1, 0, 2).reshape(128, -1))

    in_maps = []
    for c in range(NCORES):
        kv = c // 2
        in_maps.append({
            "xh": xh,
            "wqh": pmaj(wqT[:, c * HPC * D:(c + 1) * HPC * D]),
            "wkh": pmaj(wkT[:, kv * D:(kv + 1) * D]),
            "wvh": pmaj(wvT[:, kv * D:(kv + 1) * D]),
            "woh": pmaj(woT[c * HPC * D:(c + 1) * HPC * D, :]),
            "kw": np.ascontiguousarray(key_weights[c * HPC:(c + 1) * HPC]),
        })
    return in_maps


def _concat_args(ex, in_maps):
    concat_in = [
        np.concatenate([np.asarray(in_maps[c][name]) for c in range(NCORES)], axis=0)
        for name in ex["in_names"][:ex["n_params"]]
    ]
    zeros = [
        np.zeros((NCORES * av.shape[0], *av.shape[1:]), av.dtype)
        for av in ex["out_avals"]
    ]
    return concat_in + zeros


def kernel(x, wq, wk, wv, wo, key_weights):
    ex = _get_exec()
    in_maps = _prep_in_maps(x, wq, wk, wv, wo, key_weights)
    args = _concat_args(ex, in_maps)
    out_arrs = ex["sharded"](*args)
    total = np.asarray(out_arrs[0]).reshape(NCORES, SQ, HID).sum(axis=0, dtype=np.float32)
    return total.reshape(B, S, HID)

